# revision 5
# baseline (speedup 1.0000x reference)
"""Trainium2 Bass kernel for nn_ExplodedLogit (topk_masking).

Reference computation (x (512,256) f32, W (1,256) f32, b (1,) f32):
    scores = x @ W.T + b                                  (512, 1)
    idx    = argmax(scores)
    mask   = ones(512) with log(1e-46) at idx
    block  = scores * mask[None, :]                       (512, 512)
    out    = concat([scores, tile(block, (1, 512))], 1)   (512, 262145)

Sharding: the 512 identical block repetitions are split across 8
NeuronCores, 64 reps each -> per-core "rep" output (512, 32768) = 64 MB
(memory-bound: this is an HBM-write problem). Every core runs the
identical program: scores/argmax/mask are recomputed redundantly (tiny),
and the per-core slice is materialized with fan-out DMAs that read a
small SBUF block through a step-0 (broadcast) access-pattern dim.

Row layout: r = 4p + t (p = partition 0..127, t = 0..3) so the x load is
128 contiguous 4KB descriptors (one DMA).  Mask path: global max via a
tiny PE transpose chain (no GPSIMD custom ops on the critical path — the
Q7 library swap costs ~7us), block-diagonal spread by a DVE multiply
against a constant selector, and ONE single-pass bf16 matmul broadcasts
the mask to all partitions (bf16(log 1e-46) = -106.0 -> 7.6e-4 relative
output error, far under tolerance).

End-of-stream rendezvous: the runtime's completion handling for the
first core to finish throttles every still-streaming core's DMA to
~20 GB/s, so a ~2us finish skew costs the last core ~40us.  Each core
therefore writes a marker after its main stream (same HWDGE ring ->
drains after the stream), AllGathers the markers (cross-core barrier),
and only then writes the final TAIL columns — all cores finish within
~1us and the crawl window vanishes.
"""

import math

import numpy as np

import concourse.bacc as bacc
import concourse.bass_utils as _bass_utils
import concourse.mybir as mybir
import concourse.tile as tile
from concourse.bass_utils import run_bass_kernel_spmd

# If profiling is enabled via env (BASS_TRACE), a failed artifact upload
# must not take down the run — fall back to the local tmpdir.
_orig_upload = _bass_utils.upload_artifacts


def _safe_upload(tmpdir):
    try:
        return _orig_upload(tmpdir)
    except Exception:
        return tmpdir


_bass_utils.upload_artifacts = _safe_upload

F32 = mybir.dt.float32
BF16 = mybir.dt.bfloat16
MASK_VAL = float(np.float32(math.log(1e-46)))  # ~ -105.9189

T = 512        # tracks (rows)
F = 256        # features
P = 128        # SBUF partitions
TPP = T // P   # 4 rows per partition (r = 4p + t)
NREP = 512     # total block repetitions in the full output
NCORES = 8
RPC = NREP // NCORES   # 64 reps per core
R = 8                  # reps materialized in SBUF
G = RPC // R           # step-0 groups per fan-out DMA
TAIL = 64              # gated final piece (columns of the last rep)
RENDEZVOUS = True


def _build():
    nc = bacc.Bacc("TRN2", target_bir_lowering=False, debug=False,
                   num_devices=NCORES)
    x = nc.dram_tensor("x", [T, F], F32, kind="ExternalInput")
    W = nc.dram_tensor("W", [1, F], F32, kind="ExternalInput")
    b = nc.dram_tensor("b", [1, 1], F32, kind="ExternalInput")
    rep_out = nc.dram_tensor("rep", [T, RPC * T], F32, kind="ExternalOutput")
    scores_out = nc.dram_tensor("scores", [T, 1], F32, kind="ExternalOutput")
    cc_in = nc.dram_tensor("cc_in", [1, 1], F32, kind="Internal")
    cc_out = nc.dram_tensor("cc_out", [1, NCORES], F32, kind="Internal")

    with tile.TileContext(nc) as tc:
        with (
            tc.tile_pool(name="sbuf", bufs=1) as sbuf_pool,
            tc.tile_pool(name="psum", bufs=1, space="PSUM") as psum_pool,
        ):
            _emit(nc, x[:], W[:], b[:], rep_out[:], scores_out[:],
                  cc_in[:], cc_out[:], sbuf_pool, psum_pool)
    nc.compile()
    return nc


def _emit(nc, x, W, b, rep_out, scores_out, cc_in, cc_out,
          sbuf_pool, psum_pool):
    x_sb = sbuf_pool.tile([P, TPP * F], F32)     # x[4p+t, f] at [p, t*F+f]
    w_sb = sbuf_pool.tile([P, F], F32)
    b_sb = sbuf_pool.tile([P, 1], F32)
    tmp_v = sbuf_pool.tile([P, 2 * F], F32)      # DVE scratch (chunks 0,1)
    tmp_g = sbuf_pool.tile([P, 2 * F], F32)      # GPSIMD scratch (chunks 2,3)
    sc_sb = sbuf_pool.tile([P, TPP], F32)        # scores: s[4p+t] at [p,t]
    ones_sb = sbuf_pool.tile([P, P], F32)
    id_sb = sbuf_pool.tile([P, P], F32)          # 128x128 identity
    ones4b_sb = sbuf_pool.tile([TPP, P], BF16)   # lhsT for mask broadcast
    onesk_sb = sbuf_pool.tile([TPP, P * TPP], BF16)
    sel4b_sb = sbuf_pool.tile([TPP, P * TPP], BF16)  # block-diag selector
    mx4_sb = sbuf_pool.tile([TPP, 1], F32)       # per-chunk max
    g1_sb = sbuf_pool.tile([1, 1], F32)          # global max (partition 0)
    gm4_sb = sbuf_pool.tile([TPP, 1], F32)       # global max, partitions 0-3
    ind4_sb = sbuf_pool.tile([TPP, P], F32)      # argmax one-hot, chunked
    mask4b_sb = sbuf_pool.tile([TPP, P], BF16)   # mask values, chunked
    msk4d_sb = sbuf_pool.tile([TPP, P * TPP], BF16)  # block-diag spread
    rep_sb = sbuf_pool.tile([P, TPP * R * T], F32)
    marker_sb = sbuf_pool.tile([1, 1], F32)
    ccb_sb = sbuf_pool.tile([P, 1], F32)
    tail_sb = sbuf_pool.tile([P, TAIL], F32)

    sT_ps = psum_pool.tile([TPP, P], F32)
    mxT_ps = psum_pool.tile([1, TPP], F32)
    gm4_ps = psum_pool.tile([TPP, 1], F32)
    mask_ps = psum_pool.tile([P, T], F32)

    # ---- constants (overlap with the x load) ----
    nc.vector.memset(ones_sb[:], 1.0)
    nc.vector.memset(ones4b_sb[:], 1.0)
    nc.vector.memset(onesk_sb[:], 1.0)
    nc.vector.memset(marker_sb[:], 0.0)
    # identity: keep ones where (col - row) == 0
    nc.gpsimd.affine_select(
        id_sb[:], ones_sb[:], [[1, P]], mybir.AluOpType.is_equal, 0.0,
        base=0, channel_multiplier=-1,
    )
    # block-diag selector: sel4[k, p', t'] = 1 iff t' == k
    # (output column j = 4p' + t' matches row order r = 4p + t)
    nc.gpsimd.affine_select(
        sel4b_sb[:].rearrange("k (m t) -> k m t", t=TPP),
        onesk_sb[:].rearrange("k (m t) -> k m t", t=TPP),
        [[0, P], [1, TPP]], mybir.AluOpType.is_equal, 0.0,
        base=0, channel_multiplier=-1,
    )

    # ---- loads (all on the SP ring, x first; scalar keeps its ACT table
    # load off the DMA path) ----
    nc.sync.dma_start(x_sb[:], x.rearrange("(p t) f -> p (t f)", p=P))
    nc.sync.dma_start(w_sb[:], W.broadcast_to((P, F)))
    nc.sync.dma_start(b_sb[:], b.broadcast_to((P, 1)))

    # ---- scores: s[4p+t] = b + sum_f x[4p+t,f] * W[f] ----
    # muls for chunks 2,3 on GPSIMD in parallel with DVE (tensor_tensor
    # never grabs the shared DVE/GPSIMD port pair); reduces are DVE-only
    for t in (2, 3):
        o = (t - 2) * F
        nc.gpsimd.tensor_mul(
            tmp_g[:, o:o + F], x_sb[:, t * F:(t + 1) * F], w_sb[:]
        )
    for t in (0, 1):
        o = t * F
        nc.vector.tensor_mul(
            tmp_v[:, o:o + F], x_sb[:, t * F:(t + 1) * F], w_sb[:]
        )
        nc.vector.reduce_sum(
            sc_sb[:, t:t + 1], tmp_v[:, o:o + F], axis=mybir.AxisListType.X,
        )
    for t in (2, 3):
        o = (t - 2) * F
        nc.vector.reduce_sum(
            sc_sb[:, t:t + 1], tmp_g[:, o:o + F], axis=mybir.AxisListType.X,
        )

    # ---- transpose PRE-bias scores to the free dim: sT[t, p] = s[4p+t]
    # (argmax is shift-invariant; bias is added to sc_sb in parallel) ----
    nc.tensor.matmul(sT_ps[:], lhsT=sc_sb[:], rhs=id_sb[:])
    nc.vector.tensor_scalar_add(sc_sb[:], sc_sb[:], b_sb[:, 0:1])
    # external scores output (off the critical path, ACT ring)
    nc.scalar.dma_start(
        scores_out.rearrange("(p t) one -> p (t one)", p=P), sc_sb[:]
    )

    # ---- global max via tiny PE hops: [4,1] -> [1,4] -> [1,1] -> [4,1]
    nc.vector.reduce_max(mx4_sb[:], sT_ps[:], axis=mybir.AxisListType.X)
    nc.tensor.matmul(mxT_ps[:], lhsT=mx4_sb[:], rhs=id_sb[0:TPP, 0:TPP])
    nc.vector.reduce_max(g1_sb[:], mxT_ps[:], axis=mybir.AxisListType.X)
    nc.tensor.matmul(gm4_ps[:], lhsT=ones_sb[0:1, 0:TPP], rhs=g1_sb[:])
    nc.vector.tensor_scalar_add(gm4_sb[:], gm4_ps[:], 0.0)

    # ---- mask on partitions 0..3: ind = (s == gmax); m = 1 + ind*(MV-1)
    nc.vector.tensor_scalar(
        ind4_sb[:], sT_ps[:], gm4_sb[:, 0:1], None,
        mybir.AluOpType.is_equal,
    )
    nc.vector.tensor_scalar(
        mask4b_sb[:], ind4_sb[:], MASK_VAL - 1.0, 1.0,
        mybir.AluOpType.mult, mybir.AluOpType.add,
    )
    # block-diagonal spread: msk4d[k, p', t'] = mask4[k, p'] * sel4[k, p', t']
    nc.vector.tensor_mul(
        msk4d_sb[:].rearrange("k (m t) -> k m t", t=TPP),
        sel4b_sb[:].rearrange("k (m t) -> k m t", t=TPP),
        mask4b_sb[:].unsqueeze(2).broadcast_to((TPP, P, TPP)),
    )
    # broadcast to all 128 partitions: ONE single-pass bf16 matmul
    nc.tensor.matmul(mask_ps[:], lhsT=ones4b_sb[:], rhs=msk4d_sb[:])

    # ---- fill rep_sb: R copies of each row's block slice ----
    # rep_sb[p, (t*R+r)*T + c] = sc[p,t] * mask[c]   (mask read from PSUM)
    # t=0 gates the first fan-out DMA: fill its halves on DVE and ACT in
    # parallel and write them with separate DMAs so streaming starts after
    # half a fill. t=1..3 overlap with streaming anyway.
    h = R // 2
    nc.vector.tensor_scalar(
        rep_sb[:, 0:h * T].rearrange("p (r c) -> p r c", c=T),
        mask_ps.unsqueeze(1).broadcast_to((P, h, T)),
        sc_sb[:, 0:1], None, mybir.AluOpType.mult,
    )
    nc.scalar.activation(
        rep_sb[:, h * T:R * T].rearrange("p (r c) -> p r c", c=T),
        mask_ps.unsqueeze(1).broadcast_to((P, h, T)),
        mybir.ActivationFunctionType.Copy,
        scale=sc_sb[:, 0:1],
    )
    for t in range(1, TPP):
        nc.vector.tensor_scalar(
            rep_sb[:, t * R * T:(t + 1) * R * T].rearrange(
                "p (r c) -> p r c", c=T
            ),
            mask_ps.unsqueeze(1).broadcast_to((P, R, T)),
            sc_sb[:, t:t + 1], None, mybir.AluOpType.mult,
        )

    # ---- fan-out DMAs: write each t-slot G times via a step-0 src dim ----
    out_v = rep_out.rearrange("(p t) (g u) -> t p g u", p=P, u=R * T)
    # t=0 in rep-halves so the first write only waits for half a fill
    for half in range(2):
        src = (
            rep_sb[:, half * h * T:(half + 1) * h * T]
            .unsqueeze(1)
            .broadcast_to((P, G, h * T))
        )
        dst = out_v[0][:, :, half * h * T:(half + 1) * h * T]
        nc.sync.dma_start(dst, src)
    for t in (1, 2):
        src = (
            rep_sb[:, t * R * T:(t + 1) * R * T]
            .unsqueeze(1)
            .broadcast_to((P, G, R * T))
        )
        nc.sync.dma_start(out_v[t], src)

    t3 = 3 * R * T
    if not RENDEZVOUS:
        src = rep_sb[:, t3:t3 + R * T].unsqueeze(1).broadcast_to((P, G, R * T))
        nc.sync.dma_start(out_v[3], src)
        return

    # t=3 splits so a TAIL-column final piece can be gated on the barrier
    srcA = (
        rep_sb[:, t3:t3 + R * T].unsqueeze(1).broadcast_to((P, G - 1, R * T))
    )
    nc.sync.dma_start(out_v[3][:, 0:G - 1, :], srcA)
    srcB = rep_sb[:, t3:t3 + R * T - TAIL].unsqueeze(1)
    nc.sync.dma_start(out_v[3][:, G - 1:G, 0:R * T - TAIL], srcB)

    # rendezvous: marker drains after the stream (same HWDGE ring), the
    # AllGather completes only when every core's marker landed, and the
    # final piece is data-gated on the gathered result (markers are 0.0,
    # so the gate add is numerically exact).
    nc.sync.dma_start(cc_in, marker_sb[:])
    nc.gpsimd.collective_compute(
        "AllGather", mybir.AluOpType.bypass,
        [list(range(NCORES))], [cc_in], [cc_out],
    )
    nc.sync.dma_start(ccb_sb[:], cc_out[0:1, 0:1].broadcast_to((P, 1)))
    nc.vector.tensor_scalar_add(
        tail_sb[:], rep_sb[:, t3 + R * T - TAIL:t3 + R * T], ccb_sb[:, 0:1]
    )
    nc.sync.dma_start(
        out_v[3][:, G - 1:G, R * T - TAIL:R * T], tail_sb[:].unsqueeze(1)
    )


_NC_CACHE = None


def _get_nc():
    global _NC_CACHE
    if _NC_CACHE is None:
        _NC_CACHE = _build()
    return _NC_CACHE


def _run(x, W, b, **run_kwargs):
    nc = _get_nc()
    in_map = {
        "x": np.ascontiguousarray(np.asarray(x, dtype=np.float32)),
        "W": np.ascontiguousarray(np.asarray(W, dtype=np.float32)).reshape(1, F),
        "b": np.ascontiguousarray(np.asarray(b, dtype=np.float32)).reshape(1, 1),
    }
    # The device pool occasionally throws a transient
    # NRT_EXEC_UNIT_UNRECOVERABLE on dispatch; a retry lands cleanly.
    last_err = None
    for attempt in range(3):
        try:
            return run_bass_kernel_spmd(
                nc,
                [dict(in_map) for _ in range(NCORES)],
                core_ids=list(range(NCORES)),
                **run_kwargs,
            )
        except Exception as e:  # noqa: BLE001
            last_err = e
            import time
            time.sleep(2.0 * (attempt + 1))
            try:
                import jax
                jax.clear_caches()
                jax.clear_backends()
            except Exception:
                pass
    raise last_err


def kernel(x, W, b):
    res = _run(x, W, b)
    outs = res.results
    full = np.empty((T, 1 + NREP * T), dtype=np.float32)
    full[:, 0:1] = outs[0]["scores"]
    for c in range(NCORES):
        full[:, 1 + c * RPC * T: 1 + (c + 1) * RPC * T] = outs[c]["rep"]
    return full
